# revision 12
# baseline (speedup 1.0000x reference)
"""GQA attention (B=1, T=2048, C=2048, 16 Q heads / 4 KV heads, head_dim=128)
with RoPE, logit softcap 50, causal mask, softmax, output projection.

Sharding: 16 Q-heads over 8 NeuronCores (2 Q-heads + their single KV head per
core). Each core computes its partial output projection over its 2 heads; the
host sums the 8 partials (the post-projection all-reduce).

Device layout (per core):
  xT  [C, T] bf16 in SBUF (C on partitions, 16 chunks)
  Q^T [k, s] per head, K^T [k, d]    from matmul(lhsT=W chunk, rhs=xT chunk)
  RoPE applied in [k, s] layout: rot(q) = Rm @ q via a sign-permutation matmul,
    then q*cosT + rot*sinT on VectorE.
  S^T [d, s] = matmul(lhsT=K^T block, rhs=Q^T chunk)  (so the post-softmax
    matrix is already the PV lhsT -> no transpose of P needed)
  softcap+mask+softmax: tanh on ScalarE (scale 1/(50*sqrt(128))), triangular
    -40 bias added on diagonal 128-blocks, exp on ScalarE (scale 50). Softcap
    bounds logits to +-50 so no max-subtraction is needed.
  PV: O_aug[s,129] = matmul(lhsT=P^T slice, rhs=V_aug) where V_aug has a ones
    column -> column 128 accumulates the softmax denominator for free.
  normalize by 1/r (per-partition scalar), transpose O via TensorE, output
    projection back to [s, m], DMA out f32.
"""

import sys

sys.path.insert(0, "/opt/trn_rl_repo")

import math
from contextlib import ExitStack

import numpy as np
import ml_dtypes

import concourse.bass as bass
import concourse.tile as tile
from concourse import bacc
from concourse import mybir
from concourse.bass_utils import run_bass_kernel_spmd
from concourse.masks import make_identity

BF16 = ml_dtypes.bfloat16
T = 2048
C = 2048
HD = 128
NQH, NKVH = 16, 4
R = NQH // NKVH  # 4
ROPE_THETA = 10000.0
SOFTCAP = 50.0
NCORES = 8

F32 = mybir.dt.float32
BF = mybir.dt.bfloat16
AFT = mybir.ActivationFunctionType

TANH_SCALE = 1.0 / (math.sqrt(float(HD)) * SOFTCAP)
MASK_BIAS = -40.0  # added to tanh output; exp scale 50 -> -2000 in the exponent

_NC_CACHE = {}


def build_nc():
    if "nc" in _NC_CACHE:
        return _NC_CACHE["nc"]
    nc = bacc.Bacc(None, target_bir_lowering=False)
    xT = nc.dram_tensor("xT", [C, T], BF, kind="ExternalInput")
    wq = nc.dram_tensor("wq", [C, 2 * HD], BF, kind="ExternalInput")
    wk = nc.dram_tensor("wk", [C, HD], BF, kind="ExternalInput")
    wv = nc.dram_tensor("wv", [C, HD], BF, kind="ExternalInput")
    wo = nc.dram_tensor("wo", [2 * HD, C], BF, kind="ExternalInput")
    cosT = nc.dram_tensor("cosT", [HD, T], BF, kind="ExternalInput")
    sinT = nc.dram_tensor("sinT", [HD, T], F32, kind="ExternalInput")
    rmT = nc.dram_tensor("rmT", [HD, HD], BF, kind="ExternalInput")
    tri = nc.dram_tensor("tri", [HD, HD], F32, kind="ExternalInput")
    out = nc.dram_tensor("out", [T, C], F32, kind="ExternalOutput")

    NCH = C // 128  # 16 contraction chunks
    NSB = T // 128  # 16 s-blocks
    NJ = T // 512  # 4 s-chunks of 512

    with tile.TileContext(nc) as tc, ExitStack() as ctx:
        consts = ctx.enter_context(tc.tile_pool(name="consts", bufs=1))
        qkv = ctx.enter_context(tc.tile_pool(name="qkv", bufs=1))
        osmall = ctx.enter_context(tc.tile_pool(name="osmall", bufs=2))
        outsb = ctx.enter_context(tc.tile_pool(name="outsb", bufs=4))
        tpool = ctx.enter_context(tc.tile_pool(name="tpool", bufs=2))
        ptpool = ctx.enter_context(tc.tile_pool(name="ptpool", bufs=1))
        # PSUM budget (8 banks): proj 2 + sg 4 + o 1 + ot 1
        ps = ctx.enter_context(tc.tile_pool(name="ps", bufs=2, space="PSUM"))
        ps_sg = ctx.enter_context(tc.tile_pool(name="ps_sg", bufs=2, space="PSUM"))
        ps_o = ctx.enter_context(tc.tile_pool(name="ps_o", bufs=1, space="PSUM"))
        ps_ot = ctx.enter_context(tc.tile_pool(name="ps_ot", bufs=1, space="PSUM"))

        ident = consts.tile([128, 128], BF, tag="ident")
        make_identity(nc, ident)
        tri_sb = consts.tile([128, 128], F32, tag="tri")
        nc.sync.dma_start(out=tri_sb, in_=tri[:, :])
        wo_sb = consts.tile([128, 2, C], BF, tag="wo")
        for h in range(2):
            nc.sync.dma_start(out=wo_sb[:, h, :], in_=wo[h * 128:(h + 1) * 128, :])

        QT = qkv.tile([128, 2, T], BF, tag="QT")
        KT = qkv.tile([128, T], BF, tag="KT")
        Vaug = qkv.tile([128, NCH, 132], BF, tag="Vaug")
        OT = qkv.tile([128, 2, T], BF, tag="OT")
        nc.vector.memset(Vaug[:, :, 128:129], 1.0)

        pt_tiles = {}

        def attn_scores(J):
            n_i = 4 * J + 4
            PT = ptpool.tile([128, 2, n_i, 512], BF, tag="pt", name=f"PT{J}")
            pt_tiles[J] = PT
            for i in range(n_i):
                b = i - 4 * J
                c0 = 256 if b >= 2 else 0  # cols below are never consumed
                csl = slice(c0, 512)
                sg = ps_sg.tile([128, 2, 512], F32, tag="sg")
                for h in range(2):
                    nc.tensor.matmul(
                        sg[:, h, csl],
                        KT[:, i * 128:(i + 1) * 128],
                        QT[:, h, J * 512 + c0:(J + 1) * 512],
                        start=True, stop=True,
                    )
                tt = tpool.tile([128, 2, 512], F32, tag="t")
                nc.scalar.activation(
                    tt[:, :, csl], sg[:, :, csl], AFT.Tanh, scale=TANH_SCALE
                )
                if b >= 0:  # diagonal block: apply triangular mask bias
                    dsl = slice(b * 128, (b + 1) * 128)
                    for h in range(2):
                        nc.vector.tensor_add(tt[:, h, dsl], tt[:, h, dsl], tri_sb)
                nc.scalar.activation(
                    PT[:, :, i, csl], tt[:, :, csl], AFT.Exp, scale=SOFTCAP
                )

        def attn_pv_out(J):
            PT = pt_tiles.pop(J)
            for sb_ in range(4):
                j = 4 * J + sb_
                for h in range(2):
                    po = ps_o.tile([128, 129], F32, tag="o")
                    for i in range(j + 1):
                        nc.tensor.matmul(
                            po,
                            PT[:, h, i, sb_ * 128:(sb_ + 1) * 128],
                            Vaug[:, i, 0:129],
                            start=(i == 0), stop=(i == j),
                        )
                    rinv = osmall.tile([128, 1], F32, tag="rinv")
                    nc.vector.reciprocal(rinv, po[:, 128:129])
                    on = osmall.tile([128, 128], BF, tag="on")
                    nc.vector.tensor_scalar_mul(on, po[:, 0:128], rinv)
                    pot = ps_ot.tile([128, 128], BF, tag="ot")
                    nc.tensor.transpose(pot, on, ident)
                    nc.vector.tensor_copy(OT[:, h, j * 128:(j + 1) * 128], pot)
                # fused output projection for this s-block
                for mch in range(NJ):
                    p = ps.tile([128, 512], F32, tag="proj")
                    for h in range(2):
                        nc.tensor.matmul(
                            p,
                            OT[:, h, j * 128:(j + 1) * 128],
                            wo_sb[:, h, mch * 512:(mch + 1) * 512],
                            start=(h == 0), stop=(h == 1),
                        )
                    ob = outsb.tile([128, 512], F32, tag="ob")
                    nc.vector.tensor_copy(ob, p)
                    nc.sync.dma_start(
                        out=out[j * 128:(j + 1) * 128, mch * 512:(mch + 1) * 512],
                        in_=ob,
                    )

        with tc.tile_pool(name="ph1", bufs=1) as ph1, \
             tc.tile_pool(name="work", bufs=3) as work, \
             tc.tile_pool(name="ropet", bufs=2) as ropet:
            rm_sb = ph1.tile([128, 128], BF, tag="rm")
            nc.sync.dma_start(out=rm_sb, in_=rmT[:, :])
            cos_sb = ph1.tile([128, T], BF, tag="cos")
            nc.sync.dma_start(out=cos_sb, in_=cosT[:, :])
            sin_sb = ph1.tile([128, T], F32, tag="sin")
            nc.sync.dma_start(out=sin_sb, in_=sinT[:, :])
            wq_sb = ph1.tile([128, NCH, 2 * HD], BF, tag="wq")
            wk_sb = ph1.tile([128, NCH, HD], BF, tag="wk")
            wv_sb = ph1.tile([128, NCH, HD], BF, tag="wv")
            x_sb = ph1.tile([128, NCH, T], BF, tag="x")
            for c in range(NCH):
                nc.sync.dma_start(out=x_sb[:, c, :], in_=xT[c * 128:(c + 1) * 128, :])
                nc.sync.dma_start(out=wk_sb[:, c, :], in_=wk[c * 128:(c + 1) * 128, :])
                nc.sync.dma_start(out=wq_sb[:, c, :], in_=wq[c * 128:(c + 1) * 128, :])
                nc.sync.dma_start(out=wv_sb[:, c, :], in_=wv[c * 128:(c + 1) * 128, :])

            # one 512-wide output chunk of a projection + rope, fused
            def proj_chunk(w_slice_fn, ch, dst):
                sl = slice(ch * 512, (ch + 1) * 512)
                p = ps.tile([128, 512], F32, tag="proj")
                for c in range(NCH):
                    nc.tensor.matmul(
                        p, w_slice_fn(c), x_sb[:, c, sl],
                        start=(c == 0), stop=(c == NCH - 1),
                    )
                z = work.tile([128, 512], BF, tag="z")
                nc.scalar.copy(z, p)
                pr = ps.tile([128, 512], F32, tag="proj")
                nc.tensor.matmul(pr, rm_sb, z, start=True, stop=True)
                m2 = ropet.tile([128, 512], F32, tag="m2")
                nc.vector.tensor_mul(m2, pr, sin_sb[:, sl])
                m1 = ropet.tile([128, 512], F32, tag="m1")
                nc.vector.tensor_mul(m1, z, cos_sb[:, sl])
                nc.vector.tensor_add(dst[:, sl], m1, m2)

            def v_chunk(ch):
                sl = slice(ch * 512, (ch + 1) * 512)
                p = ps.tile([128, 512], F32, tag="proj")
                for c in range(NCH):
                    nc.tensor.matmul(
                        p, wv_sb[:, c, :], x_sb[:, c, sl],
                        start=(c == 0), stop=(c == NCH - 1),
                    )
                z = work.tile([128, 512], BF, tag="z")
                nc.scalar.copy(z, p)
                for b in range(4):
                    dt = 4 * ch + b
                    pv = ps_ot.tile([128, 128], BF, tag="ot")
                    nc.tensor.transpose(pv, z[:, b * 128:(b + 1) * 128], ident)
                    nc.vector.tensor_copy(Vaug[:, dt, 0:128], pv)

            # deep pipeline over 512-chunks: PV of the previous chunk first
            # (so exp(J) never waits long for the PT slot), then the K/Q
            # chunks this J needs, its scores, then its V chunk.
            for ch in range(NJ):
                if ch >= 1:
                    attn_pv_out(ch - 1)
                proj_chunk(lambda c: wk_sb[:, c, :], ch, KT)
                proj_chunk(lambda c: wq_sb[:, c, 0:HD], ch, QT[:, 0, :])
                proj_chunk(lambda c: wq_sb[:, c, HD:2 * HD], ch, QT[:, 1, :])
                attn_scores(ch)
                v_chunk(ch)
            attn_pv_out(3)

    nc.finalize()
    _NC_CACHE["nc"] = nc
    return nc


def _rope_tables():
    fraction = np.arange(0, HD, 2, dtype=np.float64) / HD
    timescale = ROPE_THETA ** fraction
    inv = 1.0 / timescale
    sin_inp = np.outer(np.arange(T, dtype=np.float64), inv)
    sin_inp = np.concatenate([sin_inp, sin_inp], axis=-1)  # [T, HD]
    sin = np.sin(sin_inp).astype(np.float32)
    cos = np.cos(sin_inp).astype(np.float32)
    return cos.T.copy(), sin.T.copy()  # [HD, T]


def _numpy_fallback(x, mask, q_kernel, k_kernel, v_kernel, out_kernel):
    # generic-mask reference path (host, f32) - only used if the mask is not
    # the standard causal mask.
    b, t, c = x.shape
    q = np.einsum("bsm,mrhk->brhsk", x, q_kernel)
    k = np.einsum("bdm,mhk->bhdk", x, k_kernel)
    v = np.einsum("bdm,mhv->bhdv", x, v_kernel)
    cosT, sinT = _rope_tables()
    cos, sin = cosT.T, sinT.T  # [T, HD]

    def rot(z):
        z1, z2 = np.split(z, 2, axis=-1)
        return np.concatenate([-z2, z1], axis=-1)

    q = q * cos[None, None, None] + rot(q) * sin[None, None, None]
    k = k * cos[None, None] + rot(k) * sin[None, None]
    s = np.einsum("brhsk,bhdk->brhsd", q, k) / np.sqrt(np.float32(HD))
    s = np.tanh(s / SOFTCAP) * SOFTCAP
    m = mask[:, None]  # [B,1,1,T,T]
    s = np.where(m, s, -np.inf)
    s = s - s.max(axis=-1, keepdims=True)
    e = np.exp(s)
    p = e / e.sum(axis=-1, keepdims=True)
    p = np.where(m, p, 0.0)
    qkv = np.einsum("brhsd,bhdv->brhsv", p, v)
    return np.einsum("brhsv,rhvm->bsm", qkv, out_kernel).astype(np.float32)


def kernel(x, mask, q_kernel, k_kernel, v_kernel, out_kernel, _trace=False):
    x = np.asarray(x)
    mask = np.asarray(mask)
    causal = bool(
        np.array_equal(mask[0, 0], np.tril(np.ones((T, T), dtype=bool)))
    )
    if not causal:
        return _numpy_fallback(x, mask, q_kernel, k_kernel, v_kernel, out_kernel)

    q_kernel = np.asarray(q_kernel, dtype=np.float32)
    k_kernel = np.asarray(k_kernel, dtype=np.float32)
    v_kernel = np.asarray(v_kernel, dtype=np.float32)
    out_kernel = np.asarray(out_kernel, dtype=np.float32)

    xT = np.ascontiguousarray(x[0].T).astype(BF16)
    cosT, sinT = _rope_tables()
    cosT_bf = cosT.astype(BF16)
    rm = np.zeros((HD, HD), dtype=np.float32)
    for kk in range(HD // 2):
        rm[kk, kk + HD // 2] = -1.0
    for kk in range(HD // 2, HD):
        rm[kk, kk - HD // 2] = 1.0
    rmT = np.ascontiguousarray(rm.T).astype(BF16)
    dl = np.arange(128)[:, None]
    sl = np.arange(128)[None, :]
    tri = np.where(dl <= sl, 0.0, MASK_BIAS).astype(np.float32)

    in_maps = []
    for core in range(NCORES):
        h = core // 2
        r0 = (core % 2) * 2
        wq_c = np.ascontiguousarray(
            q_kernel[:, r0:r0 + 2, h, :].reshape(C, 2 * HD)
        ).astype(BF16)
        wk_c = np.ascontiguousarray(k_kernel[:, h, :]).astype(BF16)
        wv_c = np.ascontiguousarray(v_kernel[:, h, :]).astype(BF16)
        wo_c = np.ascontiguousarray(
            out_kernel[r0:r0 + 2, h, :, :].reshape(2 * HD, C)
        ).astype(BF16)
        in_maps.append({
            "xT": xT, "wq": wq_c, "wk": wk_c, "wv": wv_c, "wo": wo_c,
            "cosT": cosT_bf, "sinT": sinT, "rmT": rmT, "tri": tri,
        })

    nc = build_nc()
    res = run_bass_kernel_spmd(
        nc, in_maps, core_ids=list(range(NCORES)), trace=_trace
    )
    total = np.zeros((T, C), dtype=np.float32)
    for om in res.results:
        total += om["out"]
    out = total[None]
    if _trace:
        return out, res
    return out


# revision 14
# speedup vs baseline: 1.3830x; 1.3830x over previous
"""GQA attention (B=1, T=2048, C=2048, 16 Q heads / 4 KV heads, head_dim=128)
with RoPE, logit softcap 50, causal mask, softmax, output projection.

Sharding: 16 Q-heads over 8 NeuronCores (2 Q-heads + their single KV head per
core). Each core computes its partial output projection over its 2 heads; the
host sums the 8 partials (the post-projection all-reduce).

Device layout (per core):
  xT  [C, T] bf16 in SBUF (C on partitions, 16 chunks)
  Q^T [k, s] per head, K^T [k, d]    from matmul(lhsT=W chunk, rhs=xT chunk)
  RoPE applied in [k, s] layout: rot(q) = Rm @ q via a sign-permutation matmul,
    then q*cosT + rot*sinT on VectorE.
  S^T [d, s] = matmul(lhsT=K^T block, rhs=Q^T chunk)  (so the post-softmax
    matrix is already the PV lhsT -> no transpose of P needed)
  softcap+mask+softmax: tanh on ScalarE (scale 1/(50*sqrt(128))), triangular
    -40 bias added on diagonal 128-blocks, exp on ScalarE (scale 50). Softcap
    bounds logits to +-50 so no max-subtraction is needed.
  PV: O_aug[s,129] = matmul(lhsT=P^T slice, rhs=V_aug) where V_aug has a ones
    column -> column 128 accumulates the softmax denominator for free.
  normalize by 1/r (per-partition scalar), transpose O via TensorE, output
    projection back to [s, m], DMA out f32.
"""

import sys

sys.path.insert(0, "/opt/trn_rl_repo")

import math
from contextlib import ExitStack

import numpy as np
import ml_dtypes

import concourse.bass as bass
import concourse.tile as tile
from concourse import bacc
from concourse import mybir
from concourse.bass_utils import run_bass_kernel_spmd
from concourse.masks import make_identity

BF16 = ml_dtypes.bfloat16
T = 2048
C = 2048
HD = 128
NQH, NKVH = 16, 4
R = NQH // NKVH  # 4
ROPE_THETA = 10000.0
SOFTCAP = 50.0
NCORES = 8

F32 = mybir.dt.float32
BF = mybir.dt.bfloat16
AFT = mybir.ActivationFunctionType

TANH_SCALE = 1.0 / (math.sqrt(float(HD)) * SOFTCAP)
MASK_BIAS = -40.0  # added to tanh output; exp scale 50 -> -2000 in the exponent

_NC_CACHE = {}


def build_nc():
    if "nc" in _NC_CACHE:
        return _NC_CACHE["nc"]
    nc = bacc.Bacc(None, target_bir_lowering=False)
    xT = nc.dram_tensor("xT", [C, T], BF, kind="ExternalInput")
    wq = nc.dram_tensor("wq", [C, 2 * HD], BF, kind="ExternalInput")
    wk = nc.dram_tensor("wk", [C, HD], BF, kind="ExternalInput")
    wv = nc.dram_tensor("wv", [C, HD], BF, kind="ExternalInput")
    wo = nc.dram_tensor("wo", [2 * HD, C], BF, kind="ExternalInput")
    cosT = nc.dram_tensor("cosT", [HD, T], BF, kind="ExternalInput")
    sinT = nc.dram_tensor("sinT", [HD, T], F32, kind="ExternalInput")
    rmT = nc.dram_tensor("rmT", [HD, HD], BF, kind="ExternalInput")
    tri = nc.dram_tensor("tri", [HD, HD], F32, kind="ExternalInput")
    out = nc.dram_tensor("out", [T, C], F32, kind="ExternalOutput")

    NCH = C // 128  # 16 contraction chunks
    NSB = T // 128  # 16 s-blocks
    NJ = T // 512  # 4 s-chunks of 512

    with tile.TileContext(nc) as tc, ExitStack() as ctx:
        consts = ctx.enter_context(tc.tile_pool(name="consts", bufs=1))
        qkv = ctx.enter_context(tc.tile_pool(name="qkv", bufs=1))
        osmall = ctx.enter_context(tc.tile_pool(name="osmall", bufs=2))
        outsb = ctx.enter_context(tc.tile_pool(name="outsb", bufs=2))
        tpool = ctx.enter_context(tc.tile_pool(name="tpool", bufs=2))
        ptpool = []
        # PSUM budget (8 banks): proj 2 + sg 4 + o 1 + ot 1
        ps = ctx.enter_context(tc.tile_pool(name="ps", bufs=2, space="PSUM"))
        ps_sg = ctx.enter_context(tc.tile_pool(name="ps_sg", bufs=2, space="PSUM"))
        ps_o = ctx.enter_context(tc.tile_pool(name="ps_o", bufs=1, space="PSUM"))
        ps_ot = ctx.enter_context(tc.tile_pool(name="ps_ot", bufs=1, space="PSUM"))

        ident = consts.tile([128, 128], BF, tag="ident")
        make_identity(nc, ident)
        tri_sb = consts.tile([128, 128], F32, tag="tri")
        nc.sync.dma_start(out=tri_sb, in_=tri[:, :])
        wo_sb = consts.tile([128, 2, C], BF, tag="wo")
        for h in range(2):
            nc.sync.dma_start(out=wo_sb[:, h, :], in_=wo[h * 128:(h + 1) * 128, :])

        QT = qkv.tile([128, 2, T], BF, tag="QT")
        KT = qkv.tile([128, T], BF, tag="KT")
        Vaug = qkv.tile([128, NCH, 132], BF, tag="Vaug")
        OT = qkv.tile([128, 2, T], BF, tag="OT")
        nc.vector.memset(Vaug[:, :, 128:129], 1.0)

        pt_tiles = {}

        def attn_scores(J):
            n_i = 4 * J + 4
            pool_ = qkv if J < 2 else ptpool[0]
            PT = pool_.tile([128, 2, n_i, 512], BF, tag=f"pt{min(J,2)}", name=f"PT{J}")
            pt_tiles[J] = PT
            for i in range(n_i):
                b = i - 4 * J
                c0 = 256 if b >= 2 else 0  # cols below are never consumed
                csl = slice(c0, 512)
                sg = ps_sg.tile([128, 2, 512], F32, tag="sg")
                for h in range(2):
                    nc.tensor.matmul(
                        sg[:, h, csl],
                        KT[:, i * 128:(i + 1) * 128],
                        QT[:, h, J * 512 + c0:(J + 1) * 512],
                        start=True, stop=True,
                    )
                tt = tpool.tile([128, 2, 512], F32, tag="t")
                nc.scalar.activation(
                    tt[:, :, csl], sg[:, :, csl], AFT.Tanh, scale=TANH_SCALE
                )
                if b >= 0:  # diagonal block: apply triangular mask bias
                    dsl = slice(b * 128, (b + 1) * 128)
                    for h in range(2):
                        nc.vector.tensor_add(tt[:, h, dsl], tt[:, h, dsl], tri_sb)
                nc.scalar.activation(
                    PT[:, :, i, csl], tt[:, :, csl], AFT.Exp, scale=SOFTCAP
                )

        def attn_pv_out(J):
            PT = pt_tiles.pop(J)
            for sb_ in range(4):
                j = 4 * J + sb_
                for h in range(2):
                    po = ps_o.tile([128, 129], F32, tag="o")
                    for i in range(j + 1):
                        nc.tensor.matmul(
                            po,
                            PT[:, h, i, sb_ * 128:(sb_ + 1) * 128],
                            Vaug[:, i, 0:129],
                            start=(i == 0), stop=(i == j),
                        )
                    rinv = osmall.tile([128, 1], F32, tag="rinv")
                    nc.vector.reciprocal(rinv, po[:, 128:129])
                    on = osmall.tile([128, 128], BF, tag="on")
                    nc.vector.tensor_scalar_mul(on, po[:, 0:128], rinv)
                    pot = ps_ot.tile([128, 128], BF, tag="ot")
                    nc.tensor.transpose(pot, on, ident)
                    nc.vector.tensor_copy(OT[:, h, j * 128:(j + 1) * 128], pot)
                # fused output projection for this s-block; one 1MB DMA
                ob = outsb.tile([128, T], F32, tag="ob")
                for mch in range(NJ):
                    p = ps.tile([128, 512], F32, tag="proj")
                    for h in range(2):
                        nc.tensor.matmul(
                            p,
                            OT[:, h, j * 128:(j + 1) * 128],
                            wo_sb[:, h, mch * 512:(mch + 1) * 512],
                            start=(h == 0), stop=(h == 1),
                        )
                    nc.vector.tensor_copy(ob[:, mch * 512:(mch + 1) * 512], p)
                nc.sync.dma_start(out=out[j * 128:(j + 1) * 128, :], in_=ob)

        with tc.tile_pool(name="ph1", bufs=1) as ph1, \
             tc.tile_pool(name="work", bufs=3) as work, \
             tc.tile_pool(name="ropet", bufs=2) as ropet:
            rm_sb = ph1.tile([128, 128], BF, tag="rm")
            cos_sb = ph1.tile([128, T], BF, tag="cos")
            sin_sb = ph1.tile([128, T], F32, tag="sin")
            wq_sb = ph1.tile([128, NCH, 2 * HD], BF, tag="wq")
            wk_sb = ph1.tile([128, NCH, HD], BF, tag="wk")
            wv_sb = ph1.tile([128, NCH, HD], BF, tag="wv")
            x_sb = ph1.tile([128, NCH, T], BF, tag="x")
            # batched DMAs: one per tensor (x in 4 staggered quarters) --
            # DMA_DIRECT2D issue costs ~600ns each on the Sync engine, so
            # batching beats 64 tiny transfers by ~35us.
            def dma_chunks(dst, src, n, lo, hi):
                # dst [128, n(sub), cols]; src [(sub*128), cols] in DRAM
                nc.sync.dma_start(
                    out=dst[:, lo:hi, :],
                    in_=src.rearrange("(c p) s -> p c s", p=128)[:, lo:hi, :],
                )
            dma_chunks(wk_sb, wk, NCH, 0, NCH)
            dma_chunks(x_sb, xT, NCH, 0, 4)
            dma_chunks(wq_sb, wq, NCH, 0, NCH)
            dma_chunks(wv_sb, wv, NCH, 0, NCH)
            dma_chunks(x_sb, xT, NCH, 4, 8)
            nc.sync.dma_start(out=rm_sb, in_=rmT[:, :])
            nc.sync.dma_start(out=cos_sb, in_=cosT[:, :])
            nc.sync.dma_start(out=sin_sb, in_=sinT[:, :])
            dma_chunks(x_sb, xT, NCH, 8, 12)
            dma_chunks(x_sb, xT, NCH, 12, 16)

            # one 512-wide output chunk of a projection + rope, fused
            def rope_chunk(z, ch, dst):
                sl = slice(ch * 512, (ch + 1) * 512)
                pr = ps.tile([128, 512], F32, tag="proj")
                nc.tensor.matmul(pr, rm_sb, z, start=True, stop=True)
                m2 = ropet.tile([128, 512], F32, tag="m2")
                nc.vector.tensor_mul(m2, pr, sin_sb[:, sl])
                m1 = ropet.tile([128, 512], F32, tag="m1")
                nc.vector.tensor_mul(m1, z, cos_sb[:, sl])
                nc.vector.tensor_add(dst[:, sl], m1, m2)

            def proj_chunk(w_slice_fn, ch, dst):
                sl = slice(ch * 512, (ch + 1) * 512)
                p = ps.tile([128, 512], F32, tag="proj")
                for c in range(NCH):
                    nc.tensor.matmul(
                        p, w_slice_fn(c), x_sb[:, c, sl],
                        start=(c == 0), stop=(c == NCH - 1),
                    )
                z = work.tile([128, 512], BF, tag="z")
                nc.scalar.copy(z, p)
                rope_chunk(z, ch, dst)

            # K: c-outer accumulation across all 4 chunks (borrows the two
            # sg slots) so matmuls start with the first streamed x quarter.
            k0 = work.tile([128, T], BF, tag="zk", bufs=1)
            pk = [ps_sg.tile([128, 2, 512], F32, tag="sg", name=f"pk{_i}")
                  for _i in range(2)]
            for c in range(NCH):
                for ch in range(NJ):
                    nc.tensor.matmul(
                        pk[ch // 2][:, ch % 2, :],
                        wk_sb[:, c, :],
                        x_sb[:, c, ch * 512:(ch + 1) * 512],
                        start=(c == 0), stop=(c == NCH - 1),
                    )
            for half in range(2):
                nc.scalar.copy(
                    k0[:, half * 1024:(half + 1) * 1024].rearrange(
                        "p (a b) -> p a b", a=2
                    ),
                    pk[half],
                )
            for ch in range(NJ):
                rope_chunk(k0[:, ch * 512:(ch + 1) * 512], ch, KT)

            for ch in range(NJ):
                proj_chunk(lambda c: wq_sb[:, c, 0:HD], ch, QT[:, 0, :])
            for ch in range(NJ):
                proj_chunk(lambda c: wq_sb[:, c, HD:2 * HD], ch, QT[:, 1, :])

            attn_scores(0)
            attn_scores(1)

            for ch in range(NJ):
                sl = slice(ch * 512, (ch + 1) * 512)
                p = ps.tile([128, 512], F32, tag="proj")
                for c in range(NCH):
                    nc.tensor.matmul(
                        p, wv_sb[:, c, :], x_sb[:, c, sl],
                        start=(c == 0), stop=(c == NCH - 1),
                    )
                z = work.tile([128, 512], BF, tag="z")
                nc.scalar.copy(z, p)
                for b in range(4):
                    dt = 4 * ch + b
                    pv = ps_ot.tile([128, 128], BF, tag="ot")
                    nc.tensor.transpose(pv, z[:, b * 128:(b + 1) * 128], ident)
                    nc.vector.tensor_copy(Vaug[:, dt, 0:128], pv)

        ptpool.append(ctx.enter_context(tc.tile_pool(name="ptpool", bufs=2)))
        attn_pv_out(0)
        attn_scores(2)
        attn_pv_out(1)
        attn_scores(3)
        attn_pv_out(2)
        attn_pv_out(3)

    nc.finalize()
    _NC_CACHE["nc"] = nc
    return nc


def _rope_tables():
    fraction = np.arange(0, HD, 2, dtype=np.float64) / HD
    timescale = ROPE_THETA ** fraction
    inv = 1.0 / timescale
    sin_inp = np.outer(np.arange(T, dtype=np.float64), inv)
    sin_inp = np.concatenate([sin_inp, sin_inp], axis=-1)  # [T, HD]
    sin = np.sin(sin_inp).astype(np.float32)
    cos = np.cos(sin_inp).astype(np.float32)
    return cos.T.copy(), sin.T.copy()  # [HD, T]


def _numpy_fallback(x, mask, q_kernel, k_kernel, v_kernel, out_kernel):
    # generic-mask reference path (host, f32) - only used if the mask is not
    # the standard causal mask.
    b, t, c = x.shape
    q = np.einsum("bsm,mrhk->brhsk", x, q_kernel)
    k = np.einsum("bdm,mhk->bhdk", x, k_kernel)
    v = np.einsum("bdm,mhv->bhdv", x, v_kernel)
    cosT, sinT = _rope_tables()
    cos, sin = cosT.T, sinT.T  # [T, HD]

    def rot(z):
        z1, z2 = np.split(z, 2, axis=-1)
        return np.concatenate([-z2, z1], axis=-1)

    q = q * cos[None, None, None] + rot(q) * sin[None, None, None]
    k = k * cos[None, None] + rot(k) * sin[None, None]
    s = np.einsum("brhsk,bhdk->brhsd", q, k) / np.sqrt(np.float32(HD))
    s = np.tanh(s / SOFTCAP) * SOFTCAP
    m = mask[:, None]  # [B,1,1,T,T]
    s = np.where(m, s, -np.inf)
    s = s - s.max(axis=-1, keepdims=True)
    e = np.exp(s)
    p = e / e.sum(axis=-1, keepdims=True)
    p = np.where(m, p, 0.0)
    qkv = np.einsum("brhsd,bhdv->brhsv", p, v)
    return np.einsum("brhsv,rhvm->bsm", qkv, out_kernel).astype(np.float32)


def kernel(x, mask, q_kernel, k_kernel, v_kernel, out_kernel, _trace=False):
    x = np.asarray(x)
    mask = np.asarray(mask)
    causal = bool(
        np.array_equal(mask[0, 0], np.tril(np.ones((T, T), dtype=bool)))
    )
    if not causal:
        return _numpy_fallback(x, mask, q_kernel, k_kernel, v_kernel, out_kernel)

    q_kernel = np.asarray(q_kernel, dtype=np.float32)
    k_kernel = np.asarray(k_kernel, dtype=np.float32)
    v_kernel = np.asarray(v_kernel, dtype=np.float32)
    out_kernel = np.asarray(out_kernel, dtype=np.float32)

    xT = np.ascontiguousarray(x[0].T).astype(BF16)
    cosT, sinT = _rope_tables()
    cosT_bf = cosT.astype(BF16)
    rm = np.zeros((HD, HD), dtype=np.float32)
    for kk in range(HD // 2):
        rm[kk, kk + HD // 2] = -1.0
    for kk in range(HD // 2, HD):
        rm[kk, kk - HD // 2] = 1.0
    rmT = np.ascontiguousarray(rm.T).astype(BF16)
    dl = np.arange(128)[:, None]
    sl = np.arange(128)[None, :]
    tri = np.where(dl <= sl, 0.0, MASK_BIAS).astype(np.float32)

    in_maps = []
    for core in range(NCORES):
        h = core // 2
        r0 = (core % 2) * 2
        wq_c = np.ascontiguousarray(
            q_kernel[:, r0:r0 + 2, h, :].reshape(C, 2 * HD)
        ).astype(BF16)
        wk_c = np.ascontiguousarray(k_kernel[:, h, :]).astype(BF16)
        wv_c = np.ascontiguousarray(v_kernel[:, h, :]).astype(BF16)
        wo_c = np.ascontiguousarray(
            out_kernel[r0:r0 + 2, h, :, :].reshape(2 * HD, C)
        ).astype(BF16)
        in_maps.append({
            "xT": xT, "wq": wq_c, "wk": wk_c, "wv": wv_c, "wo": wo_c,
            "cosT": cosT_bf, "sinT": sinT, "rmT": rmT, "tri": tri,
        })

    nc = build_nc()
    res = run_bass_kernel_spmd(
        nc, in_maps, core_ids=list(range(NCORES)), trace=_trace
    )
    total = np.zeros((T, C), dtype=np.float32)
    for om in res.results:
        total += om["out"]
    out = total[None]
    if _trace:
        return out, res
    return out
